# revision 11
# baseline (speedup 1.0000x reference)
"""Trainium2 Bass kernel for CausalSelfAttention with block-repeated causal mask.

Problem: B=2, T=3072, C=1024, H=16 heads, d=64.
  q/k/v = x @ W{q,k,v}.T + b;  scores = q k^T / 8, masked by
  (i % 1024) >= (j % 1024) (tril(1024) tiled 3x3), softmax, y = attn @ v,
  out = y @ Wp.T + bp.

Sharding (8 cores): core i handles batch b = i//4 and heads 4*(i%4)..4*(i%4)+3
(data parallel on B, tensor parallel on heads).  Each core computes a partial
output projection (its 4 heads' contribution, no bias); the host sums the 4
partials per batch and adds bp (the TP all-reduce done at unshard time).

Device layout per core (all matmul operands bf16, fp32 accumulation):
  xT   [C=1024, T]  : x[b] transposed (c_in on partitions)
  qT,kT [128, pair, T] : head-pair-packed [d-channel, t] projections
  v    [128, ktile, 4*(64+1)] : [t, head*(d | ones)] - ones col => rowsums
  scores^T tiles [k, q] via row-packed K=64 matmuls (2 heads concurrently)
  exp on ACT (scale=1/8 folded in), no max-subtraction (|scores| ~ 2)
  attn@v: lhsT = exp(s^T) tile, rhs = v_ext -> y psum [q, 65] accumulated
  normalize by reciprocal of rowsum col, PE-transpose, output projection.
"""

import numpy as np
import ml_dtypes

import concourse.bass as bass
from concourse import bacc
import concourse.mybir as mybir
from concourse.bass import ts
from concourse.tile import TileContext
from concourse.bass_utils import run_bass_kernel_spmd
from concourse.masks import make_identity, make_upper_triangular

B, T, C, H = 2, 3072, 1024, 16
D = 64                  # head dim
NCORE = 8
HPC = 4                 # heads per core
PAIRS = 2               # head pairs per core
CHS = HPC * D           # 256 channels per core
NKT = T // 128          # 24 key tiles
NQT = T // 128          # 24 query tiles
NQP = NQT // 2          # 12 query tile-pairs
RPB = 8                 # 128-tiles per 1024 mask block
DE = D + 1              # head value cols incl. ones column

BF16 = mybir.dt.bfloat16
F32 = mybir.dt.float32

_CACHE = {}


def _build(debug=False):
    nc = bacc.Bacc()

    xT = nc.dram_tensor("xT", [C, T], BF16, kind="ExternalInput")
    wqT = nc.dram_tensor("wqT", [C, CHS], BF16, kind="ExternalInput")
    wkT = nc.dram_tensor("wkT", [C, CHS], BF16, kind="ExternalInput")
    wvT = nc.dram_tensor("wvT", [C, CHS], BF16, kind="ExternalInput")
    wpT = nc.dram_tensor("wpT", [CHS, C], BF16, kind="ExternalInput")
    bqd = nc.dram_tensor("bq", [128, PAIRS], F32, kind="ExternalInput")
    bkd = nc.dram_tensor("bk", [128, PAIRS], F32, kind="ExternalInput")
    bvd = nc.dram_tensor("bv", [128, CHS], F32, kind="ExternalInput")
    outd = nc.dram_tensor("out", [T, C], F32, kind="ExternalOutput")
    if debug:
        dbg_qT = nc.dram_tensor("dbg_qT", [128, PAIRS, T], BF16, kind="ExternalOutput")
        dbg_kT = nc.dram_tensor("dbg_kT", [128, PAIRS, T], BF16, kind="ExternalOutput")
        dbg_v = nc.dram_tensor("dbg_v", [128, NKT, HPC * DE], BF16, kind="ExternalOutput")
        dbg_esb = nc.dram_tensor("dbg_esb", [128, 1024], BF16, kind="ExternalOutput")
        dbg_py = nc.dram_tensor("dbg_py", [2, 128, 2 * DE], F32, kind="ExternalOutput")

    with TileContext(nc) as tc:
        with (
            tc.tile_pool(name="const", bufs=1) as const,
            tc.tile_pool(name="qkv", bufs=1) as qkvp,
            tc.tile_pool(name="exps", bufs=3) as expp,
            tc.tile_pool(name="ynorm", bufs=4) as ynp,
            tc.tile_pool(name="ytp", bufs=4) as ytp,
            tc.tile_pool(name="outp", bufs=2) as outp,
            tc.tile_pool(name="small", bufs=8) as smallp,
            tc.tile_pool(name="ps_s", bufs=2, space="PSUM") as ps_s,
            tc.tile_pool(name="ps_y", bufs=2, space="PSUM") as ps_y,
            tc.tile_pool(name="ps_t", bufs=1, space="PSUM") as ps_t,
            tc.tile_pool(name="ps_o", bufs=1, space="PSUM") as ps_o,
        ):
            # ---------------- constants / weights into SBUF ----------------
            xT_sb = const.tile([128, C // 128, T], BF16)
            xT_ap = xT[:, :].rearrange("(a p) t -> a p t", p=128)
            for a in range(C // 128):
                nc.sync.dma_start(out=xT_sb[:, a, :], in_=xT_ap[a])

            wq_sb = const.tile([128, C // 128, CHS], BF16)
            wk_sb = const.tile([128, C // 128, CHS], BF16)
            wv_sb = const.tile([128, C // 128, CHS], BF16)
            for a in range(C // 128):
                nc.sync.dma_start(
                    out=wq_sb[:, a, :],
                    in_=wqT[:, :].rearrange("(a p) c -> a p c", p=128)[a],
                )
                nc.sync.dma_start(
                    out=wk_sb[:, a, :],
                    in_=wkT[:, :].rearrange("(a p) c -> a p c", p=128)[a],
                )
                nc.sync.dma_start(
                    out=wv_sb[:, a, :],
                    in_=wvT[:, :].rearrange("(a p) c -> a p c", p=128)[a],
                )
            wp_sb = const.tile([128, PAIRS, C], BF16)
            for a in range(PAIRS):
                nc.sync.dma_start(
                    out=wp_sb[:, a, :],
                    in_=wpT[:, :].rearrange("(a p) c -> a p c", p=128)[a],
                )

            bq_ld = const.tile([128, PAIRS], F32)
            bk_ld = const.tile([128, PAIRS], F32)
            bv_ld = const.tile([128, CHS], F32)
            nc.sync.dma_start(out=bq_ld, in_=bqd[:, :])
            nc.sync.dma_start(out=bk_ld, in_=bkd[:, :])
            nc.sync.dma_start(out=bv_ld, in_=bvd[:, :])
            # DVE-local copies: consumers then never need a DMA sem wait
            # (walrus allows only one sync-wait on TensorScalar/TensorTensor)
            bq_sb = const.tile([128, PAIRS], F32)
            bk_sb = const.tile([128, PAIRS], F32)
            bv_sb = const.tile([128, CHS], F32)
            nc.vector.tensor_copy(bq_sb, bq_ld)
            nc.vector.tensor_copy(bk_sb, bk_ld)
            nc.vector.tensor_copy(bv_sb, bv_ld)

            ident = const.tile([128, 128], BF16)
            make_identity(nc, ident)
            # mask[k', q'] = 1 where q' >= k' (keep), else 0
            mask_sb = const.tile([128, 128], BF16)
            make_upper_triangular(nc, mask_sb, val=1.0, diag=True)

            # ---------------- q/k/v projections ----------------
            qT_sb = qkvp.tile([128, PAIRS, T], BF16)
            kT_sb = qkvp.tile([128, PAIRS, T], BF16)
            v_sb = qkvp.tile([128, NKT, HPC * DE], BF16)
            nc.vector.memset(v_sb, 1.0)  # ones columns for rowsums (DVE: keeps v bias-add single-wait)

            # qT/kT: psum[c_h(128 for the pair), t(512)] = sum_ci W^T . xT
            for pr in range(PAIRS):
                for tcn in range(T // 512):
                    pq = ps_s.tile([128, 1024], F32, name="pq", tag="sc")
                    for ci in range(C // 128):
                        nc.tensor.matmul(
                            pq[:, 0:512],
                            lhsT=wq_sb[:, ci, ts(pr, 128)],
                            rhs=xT_sb[:, ci, ts(tcn, 512)],
                            start=(ci == 0),
                            stop=(ci == C // 128 - 1),
                        )
                    for ci in range(C // 128):
                        nc.tensor.matmul(
                            pq[:, 512:1024],
                            lhsT=wk_sb[:, ci, ts(pr, 128)],
                            rhs=xT_sb[:, ci, ts(tcn, 512)],
                            start=(ci == 0),
                            stop=(ci == C // 128 - 1),
                        )
                    # ACT Identity+bias: TensorScalarPtr only allows 1 sync wait,
                    # these need PE + (DMA or DVE) waits -> use Activation opcode
                    nc.scalar.add(
                        qT_sb[:, pr, ts(tcn, 512)], pq[:, 0:512], bq_sb[:, pr : pr + 1]
                    )
                    nc.scalar.add(
                        kT_sb[:, pr, ts(tcn, 512)], pq[:, 512:1024], bk_sb[:, pr : pr + 1]
                    )

            # v: psum[t(128), c_h(256)] = sum_ci xT_tile^T . wvT
            bv_r = bv_sb.rearrange("p (h e) -> p h e", e=D)
            for th in range(NKT // 2):
                pv = ps_o.tile([128, 512], F32, name="pv", tag="po")
                for sub in range(2):
                    tt = th * 2 + sub
                    for ci in range(C // 128):
                        nc.tensor.matmul(
                            pv[:, ts(sub, 256)],
                            lhsT=xT_sb[:, ci, ts(tt, 128)],
                            rhs=wv_sb[:, ci, :],
                            start=(ci == 0),
                            stop=(ci == C // 128 - 1),
                        )
                for sub in range(2):
                    tt = th * 2 + sub
                    vt = v_sb[:, tt, :].rearrange("p (h e) -> p h e", e=DE)[:, :, 0:D]
                    pvr = pv[:, ts(sub, 256)].rearrange("p (h e) -> p h e", e=D)
                    nc.vector.tensor_add(vt, pvr, bv_r)

            if debug:
                nc.sync.dma_start(out=dbg_qT[:, :, :], in_=qT_sb)
                nc.sync.dma_start(out=dbg_kT[:, :, :], in_=kT_sb)
                nc.sync.dma_start(out=dbg_v[:, :, :], in_=v_sb)

            # ---------------- attention + output projection ----------------
            for qp in range(NQP):
                ri0 = (2 * qp) % RPB
                ri1 = ri0 + 1
                q0 = 2 * qp
                allowed = [b * RPB + r for b in range(3) for r in range(ri1 + 1)]
                allowed_q = [
                    [j for j in allowed if j % RPB <= ri0],
                    allowed,
                ]
                groups = [allowed[i : i + 2] for i in range(0, len(allowed), 2)]

                yts = []
                for hp in range(PAIRS):
                    py = [
                        ps_y.tile([128, 2 * DE], F32, name=f"py{qi}", tag="py")
                        for qi in range(2)
                    ]
                    for g in groups:
                        pscore = ps_s.tile([128, 1024], F32, name="pscore", tag="sc")
                        # scores^T [k,q] : 2 heads in row groups 0-63 / 64-127
                        for j, J in enumerate(g):
                            for h in range(2):
                                nc.tensor.matmul(
                                    pscore[:, h * 512 + j * 256 : h * 512 + j * 256 + 256],
                                    lhsT=kT_sb[h * D : (h + 1) * D, hp, ts(J, 128)],
                                    rhs=qT_sb[h * D : (h + 1) * D, hp, q0 * 128 : q0 * 128 + 256],
                                    start=True,
                                    stop=True,
                                    tile_position=(h * D, 0),
                                )
                        esb = expp.tile([128, 1024], BF16)
                        nc.scalar.activation(
                            esb, pscore, mybir.ActivationFunctionType.Exp, scale=0.125
                        )
                        # mask the two diagonal tile halves (on gpsimd, SBUF)
                        for j, J in enumerate(g):
                            r = J % RPB
                            if r in (ri0, ri1):
                                qi = 0 if r == ri0 else 1
                                for h in range(2):
                                    sl = esb[
                                        :,
                                        h * 512 + j * 256 + qi * 128 : h * 512 + j * 256 + qi * 128 + 128,
                                    ]
                                    nc.gpsimd.tensor_mul(sl, sl, mask_sb)
                        if debug and qp == 0 and hp == 0 and g is groups[0]:
                            nc.sync.dma_start(out=dbg_esb[:, :], in_=esb)
                        # attn @ v_ext -> y psum [q, d|rowsum] accumulation
                        for j, J in enumerate(g):
                            r = J % RPB
                            for h in range(2):
                                hg = hp * 2 + h
                                for qi in range(2):
                                    if qi == 0 and r == ri1:
                                        continue
                                    # start=True clears has_written for the WHOLE
                                    # bank: only the bank's first MM (h==0) may set
                                    # it, else h0's accumulation bits get wiped.
                                    # h1's first MM overwrites via cleared bits.
                                    nc.tensor.matmul(
                                        py[qi][:, h * DE : (h + 1) * DE],
                                        lhsT=esb[
                                            :,
                                            h * 512 + j * 256 + qi * 128 : h * 512 + j * 256 + qi * 128 + 128,
                                        ],
                                        rhs=v_sb[:, J, hg * DE : (hg + 1) * DE],
                                        start=(h == 0 and J == allowed_q[qi][0]),
                                        stop=(J == allowed_q[qi][-1]),
                                        skip_group_check=True,
                                    )
                    if debug and qp == 0 and hp == 0:
                        for qi in range(2):
                            pyc = outp.tile([128, 2 * DE], F32, name=f"pyc{qi}", tag="pyc")
                            nc.vector.tensor_copy(pyc, py[qi])
                            nc.sync.dma_start(out=dbg_py[qi], in_=pyc)
                    # normalize by rowsum, transpose to [d, q] layout
                    pyt = ps_t.tile([128, 256], BF16)
                    for qi in range(2):
                        for h in range(2):
                            rc = smallp.tile([128, 1], F32)
                            nc.vector.reciprocal(rc, py[qi][:, h * DE + D : h * DE + DE])
                            yn = ynp.tile([128, D], BF16)
                            nc.vector.tensor_scalar_mul(
                                yn, py[qi][:, h * DE : h * DE + D], rc
                            )
                            nc.tensor.transpose(
                                pyt[h * D : (h + 1) * D, ts(qi, 128)],
                                yn,
                                ident,
                                tile_position=(0, h * D),
                            )
                    yt = ytp.tile([128, 2, 128], BF16)
                    nc.vector.tensor_copy(
                        yt, pyt.rearrange("p (a q) -> p a q", a=2)
                    )
                    yts.append(yt)

                # output projection for the two query tiles
                for qi in range(2):
                    qt = q0 + qi
                    osb = outp.tile([128, C], F32)
                    for ch in range(2):
                        po = ps_o.tile([128, 512], F32, name="po", tag="po")
                        for hp in range(PAIRS):
                            nc.tensor.matmul(
                                po,
                                lhsT=yts[hp][:, qi, :],
                                rhs=wp_sb[:, hp, ts(ch, 512)],
                                start=(hp == 0),
                                stop=(hp == PAIRS - 1),
                            )
                        nc.vector.tensor_copy(osb[:, ts(ch, 512)], po)
                    nc.sync.dma_start(
                        out=outd[qt * 128 : (qt + 1) * 128, :], in_=osb
                    )

    nc.finalize()  # Bacc: runs compile pipeline (event-sem split, reg alloc)
    return nc


def _get_nc():
    if "nc" not in _CACHE:
        _CACHE["nc"] = _build()
    return _CACHE["nc"]


def _shard(inputs):
    bf = ml_dtypes.bfloat16
    x = np.asarray(inputs["x"], dtype=np.float32)
    Wq = np.asarray(inputs["Wq"], dtype=np.float32)
    Wk = np.asarray(inputs["Wk"], dtype=np.float32)
    Wv = np.asarray(inputs["Wv"], dtype=np.float32)
    Wp = np.asarray(inputs["Wp"], dtype=np.float32)
    bq = np.asarray(inputs["bq"], dtype=np.float32)
    bk = np.asarray(inputs["bk"], dtype=np.float32)
    bv = np.asarray(inputs["bv"], dtype=np.float32)

    in_maps = []
    for i in range(NCORE):
        b = i // 4
        j = i % 4
        hs = slice(j * CHS, (j + 1) * CHS)
        m = {
            "xT": np.ascontiguousarray(x[b].T).astype(bf),
            "wqT": np.ascontiguousarray(Wq[hs].T).astype(bf),
            "wkT": np.ascontiguousarray(Wk[hs].T).astype(bf),
            "wvT": np.ascontiguousarray(Wv[hs].T).astype(bf),
            "wpT": np.ascontiguousarray(Wp[:, hs].T).astype(bf),
            "bq": np.ascontiguousarray(bq[hs].reshape(PAIRS, 128).T),
            "bk": np.ascontiguousarray(bk[hs].reshape(PAIRS, 128).T),
            "bv": np.ascontiguousarray(np.broadcast_to(bv[hs], (128, CHS))),
        }
        in_maps.append(m)
    return in_maps


def _unshard(results, inputs):
    bp = np.asarray(inputs["bp"], dtype=np.float32)
    out = np.empty((B, T, C), dtype=np.float32)
    for b in range(B):
        acc = results[4 * b]["out"].astype(np.float32).copy()
        for j in range(1, 4):
            acc += results[4 * b + j]["out"]
        out[b] = acc + bp
    return out


def run(inputs, trace=False, debug=False):
    nc = _build(debug=True) if debug else _get_nc()
    in_maps = _shard(inputs)
    res = run_bass_kernel_spmd(nc, in_maps, list(range(NCORE)), trace=trace)
    return _unshard(res.results, inputs), res


def kernel(**inputs):
    out, _ = run(inputs, trace=False)
    return out


# revision 13
# speedup vs baseline: 1.2670x; 1.2670x over previous
"""Trainium2 Bass kernel for CausalSelfAttention with block-repeated causal mask.

Problem: B=2, T=3072, C=1024, H=16 heads, d=64.
  q/k/v = x @ W{q,k,v}.T + b;  scores = q k^T / 8, masked by
  (i % 1024) >= (j % 1024) (tril(1024) tiled 3x3), softmax, y = attn @ v,
  out = y @ Wp.T + bp.

Sharding (8 cores): core i handles batch b = i//4 and heads 4*(i%4)..4*(i%4)+3
(data parallel on B, tensor parallel on heads).  Each core computes a partial
output projection (its 4 heads' contribution, no bias); the host sums the 4
partials per batch and adds bp (the TP all-reduce done at unshard time).

Device layout per core (all matmul operands bf16, fp32 accumulation):
  xT   [C=1024, T]  : x[b] transposed (c_in on partitions)
  qT,kT [128, pair, T] : head-pair-packed [d-channel, t] projections
  v    [128, ktile, 4*(64+1)] : [t, head*(d | ones)] - ones col => rowsums
  scores^T tiles [k, q] via row-packed K=64 matmuls (2 heads concurrently)
  exp on ACT (scale=1/8 folded in), no max-subtraction (|scores| ~ 2)
  attn@v: lhsT = exp(s^T) tile, rhs = v_ext -> y psum [q, 65] accumulated
  normalize by reciprocal of rowsum col, PE-transpose, output projection.
"""

import numpy as np
import ml_dtypes

import concourse.bass as bass
from concourse import bacc
import concourse.mybir as mybir
from concourse.bass import ts
from concourse.tile import TileContext
from concourse.bass_utils import run_bass_kernel_spmd
from concourse.masks import make_identity, make_upper_triangular

B, T, C, H = 2, 3072, 1024, 16
D = 64                  # head dim
NCORE = 8
HPC = 4                 # heads per core
PAIRS = 2               # head pairs per core
CHS = HPC * D           # 256 channels per core
NKT = T // 128          # 24 key tiles
NQT = T // 128          # 24 query tiles
NQP = NQT // 2          # 12 query tile-pairs
RPB = 8                 # 128-tiles per 1024 mask block
DE = D + 1              # head value cols incl. ones column

BF16 = mybir.dt.bfloat16
F32 = mybir.dt.float32

_CACHE = {}


def _build(debug=False):
    nc = bacc.Bacc()

    xT = nc.dram_tensor("xT", [C, T], BF16, kind="ExternalInput")
    wqT = nc.dram_tensor("wqT", [C, CHS], BF16, kind="ExternalInput")
    wkT = nc.dram_tensor("wkT", [C, CHS], BF16, kind="ExternalInput")
    wvT = nc.dram_tensor("wvT", [C, CHS], BF16, kind="ExternalInput")
    wpT = nc.dram_tensor("wpT", [CHS, C], BF16, kind="ExternalInput")
    bqd = nc.dram_tensor("bq", [128, PAIRS], F32, kind="ExternalInput")
    bkd = nc.dram_tensor("bk", [128, PAIRS], F32, kind="ExternalInput")
    bvd = nc.dram_tensor("bv", [128, CHS], F32, kind="ExternalInput")
    outd = nc.dram_tensor("out", [T, C], F32, kind="ExternalOutput")
    if debug:
        dbg_qT = nc.dram_tensor("dbg_qT", [128, PAIRS, T], BF16, kind="ExternalOutput")
        dbg_kT = nc.dram_tensor("dbg_kT", [128, PAIRS, T], BF16, kind="ExternalOutput")
        dbg_v = nc.dram_tensor("dbg_v", [128, NKT, HPC * DE], BF16, kind="ExternalOutput")
        dbg_esb = nc.dram_tensor("dbg_esb", [128, 1024], BF16, kind="ExternalOutput")
        dbg_py = nc.dram_tensor("dbg_py", [2, 128, 2 * DE], F32, kind="ExternalOutput")

    with TileContext(nc) as tc:
        with (
            tc.tile_pool(name="const", bufs=1) as const,
            tc.tile_pool(name="qkv", bufs=1) as qkvp,
            tc.tile_pool(name="exps", bufs=4) as expp,
            tc.tile_pool(name="ynorm", bufs=4) as ynp,
            tc.tile_pool(name="ytp", bufs=4) as ytp,
            tc.tile_pool(name="outp", bufs=2) as outp,
            tc.tile_pool(name="small", bufs=8) as smallp,
            tc.tile_pool(name="ps_s", bufs=2, space="PSUM") as ps_s,
            tc.tile_pool(name="ps_y", bufs=2, space="PSUM") as ps_y,
            tc.tile_pool(name="ps_t", bufs=1, space="PSUM") as ps_t,
            tc.tile_pool(name="ps_o", bufs=1, space="PSUM") as ps_o,
        ):
            # ---------------- constants / weights into SBUF ----------------
            xT_sb = const.tile([128, C // 128, T], BF16)
            xT_ap = xT[:, :].rearrange("(a p) t -> a p t", p=128)
            for a in range(C // 128):
                nc.sync.dma_start(out=xT_sb[:, a, :], in_=xT_ap[a])

            wq_sb = const.tile([128, C // 128, CHS], BF16)
            wk_sb = const.tile([128, C // 128, CHS], BF16)
            wv_sb = const.tile([128, C // 128, CHS], BF16)
            for a in range(C // 128):
                nc.sync.dma_start(
                    out=wq_sb[:, a, :],
                    in_=wqT[:, :].rearrange("(a p) c -> a p c", p=128)[a],
                )
                nc.sync.dma_start(
                    out=wk_sb[:, a, :],
                    in_=wkT[:, :].rearrange("(a p) c -> a p c", p=128)[a],
                )
                nc.sync.dma_start(
                    out=wv_sb[:, a, :],
                    in_=wvT[:, :].rearrange("(a p) c -> a p c", p=128)[a],
                )
            wp_sb = const.tile([128, PAIRS, C], BF16)
            for a in range(PAIRS):
                nc.sync.dma_start(
                    out=wp_sb[:, a, :],
                    in_=wpT[:, :].rearrange("(a p) c -> a p c", p=128)[a],
                )

            bq_ld = const.tile([128, PAIRS], F32)
            bk_ld = const.tile([128, PAIRS], F32)
            bv_ld = const.tile([128, CHS], F32)
            nc.sync.dma_start(out=bq_ld, in_=bqd[:, :])
            nc.sync.dma_start(out=bk_ld, in_=bkd[:, :])
            nc.sync.dma_start(out=bv_ld, in_=bvd[:, :])
            # DVE-local copies: consumers then never need a DMA sem wait
            # (walrus allows only one sync-wait on TensorScalar/TensorTensor)
            bq_sb = const.tile([128, PAIRS], F32)
            bk_sb = const.tile([128, PAIRS], F32)
            bv_sb = const.tile([128, CHS], F32)
            nc.vector.tensor_copy(bq_sb, bq_ld)
            nc.vector.tensor_copy(bk_sb, bk_ld)
            nc.vector.tensor_copy(bv_sb, bv_ld)

            ident = const.tile([128, 128], BF16)
            make_identity(nc, ident)
            # mask[k', q'] = 1 where q' >= k' (keep), else 0
            mask_sb = const.tile([128, 128], BF16)
            make_upper_triangular(nc, mask_sb, val=1.0, diag=True)

            # ---------------- q/k/v projections ----------------
            qT_sb = qkvp.tile([128, PAIRS, T], BF16)
            # zero-padded per-head kT: full K=128 stationary for the scores
            # matmuls (rows outside the head's 64 are zero, multiplying the
            # other head's q rows by zero) - enables FWL and full-array MMs
            kTz = qkvp.tile([128, PAIRS * 2, T], BF16)
            v_sb = qkvp.tile([128, NKT, HPC * DE], BF16)
            nc.gpsimd.memset(kTz, 0.0)
            nc.vector.memset(v_sb, 1.0)  # ones columns for rowsums (DVE: keeps v bias-add single-wait)

            # qT/kT: psum[c_h(128 for the pair), t(512)] = sum_ci W^T . xT
            for pr in range(PAIRS):
                for tcn in range(T // 512):
                    pq = ps_s.tile([128, 1024], F32, name="pq", tag="sc")
                    for ci in range(C // 128):
                        nc.tensor.matmul(
                            pq[:, 0:512],
                            lhsT=wq_sb[:, ci, ts(pr, 128)],
                            rhs=xT_sb[:, ci, ts(tcn, 512)],
                            start=(ci == 0),
                            stop=(ci == C // 128 - 1),
                        )
                    for ci in range(C // 128):
                        nc.tensor.matmul(
                            pq[:, 512:1024],
                            lhsT=wk_sb[:, ci, ts(pr, 128)],
                            rhs=xT_sb[:, ci, ts(tcn, 512)],
                            start=(ci == 0),
                            stop=(ci == C // 128 - 1),
                        )
                    # ACT Identity+bias: TensorScalarPtr only allows 1 sync wait,
                    # these need PE + (DMA or DVE) waits -> use Activation opcode
                    nc.scalar.add(
                        qT_sb[:, pr, ts(tcn, 512)], pq[:, 0:512], bq_sb[:, pr : pr + 1]
                    )
                    nc.scalar.add(
                        kTz[0:D, pr * 2, ts(tcn, 512)],
                        pq[0:D, 512:1024],
                        bk_sb[0:D, pr : pr + 1],
                    )
                    nc.scalar.add(
                        kTz[D:128, pr * 2 + 1, ts(tcn, 512)],
                        pq[D:128, 512:1024],
                        bk_sb[D:128, pr : pr + 1],
                    )

            # v: psum[t(128), c_h(256)] = sum_ci xT_tile^T . wvT
            bv_r = bv_sb.rearrange("p (h e) -> p h e", e=D)
            for th in range(NKT // 2):
                pv = ps_o.tile([128, 512], F32, name="pv", tag="po")
                for sub in range(2):
                    tt = th * 2 + sub
                    for ci in range(C // 128):
                        nc.tensor.matmul(
                            pv[:, ts(sub, 256)],
                            lhsT=xT_sb[:, ci, ts(tt, 128)],
                            rhs=wv_sb[:, ci, :],
                            start=(ci == 0),
                            stop=(ci == C // 128 - 1),
                        )
                for sub in range(2):
                    tt = th * 2 + sub
                    vt = v_sb[:, tt, :].rearrange("p (h e) -> p h e", e=DE)[:, :, 0:D]
                    pvr = pv[:, ts(sub, 256)].rearrange("p (h e) -> p h e", e=D)
                    nc.vector.tensor_add(vt, pvr, bv_r)

            if debug:
                nc.sync.dma_start(out=dbg_qT[:, :, :], in_=qT_sb)
                nc.sync.dma_start(out=dbg_kT[:, :, :], in_=kTz[:, 0:2, :])
                nc.sync.dma_start(out=dbg_v[:, :, :], in_=v_sb)

            # ---------------- attention + output projection ----------------
            for qp in range(NQP):
                ri0 = (2 * qp) % RPB
                ri1 = ri0 + 1
                q0 = 2 * qp
                allowed = [b * RPB + r for b in range(3) for r in range(ri1 + 1)]
                allowed_q = [
                    [j for j in allowed if j % RPB <= ri0],
                    allowed,
                ]
                groups = [allowed[i : i + 2] for i in range(0, len(allowed), 2)]

                yts = []
                for hp in range(PAIRS):
                    py = [
                        ps_y.tile([128, 2 * DE], F32, name=f"py{qi}", tag="py")
                        for qi in range(2)
                    ]
                    for g in groups:
                        pscore = ps_s.tile([128, 1024], F32, name="pscore", tag="sc")
                        # scores^T [k,q] : 2 heads in row groups 0-63 / 64-127
                        for j, J in enumerate(g):
                            for h in range(2):
                                nc.tensor.matmul(
                                    pscore[:, h * 512 + j * 256 : h * 512 + j * 256 + 256],
                                    lhsT=kTz[:, hp * 2 + h, ts(J, 128)],
                                    rhs=qT_sb[:, hp, q0 * 128 : q0 * 128 + 256],
                                    start=True,
                                    stop=True,
                                )
                        esb = expp.tile([128, 1024], BF16)
                        nc.scalar.activation(
                            esb, pscore, mybir.ActivationFunctionType.Exp, scale=0.125
                        )
                        # mask the two diagonal tile halves (on gpsimd, SBUF)
                        for j, J in enumerate(g):
                            r = J % RPB
                            if r in (ri0, ri1):
                                qi = 0 if r == ri0 else 1
                                for h in range(2):
                                    sl = esb[
                                        :,
                                        h * 512 + j * 256 + qi * 128 : h * 512 + j * 256 + qi * 128 + 128,
                                    ]
                                    nc.vector.tensor_mul(sl, sl, mask_sb)
                        if debug and qp == 0 and hp == 0 and g is groups[0]:
                            nc.sync.dma_start(out=dbg_esb[:, :], in_=esb)
                        # attn @ v_ext -> y psum [q, d|rowsum] accumulation
                        for j, J in enumerate(g):
                            r = J % RPB
                            for h in range(2):
                                hg = hp * 2 + h
                                for qi in range(2):
                                    if qi == 0 and r == ri1:
                                        continue
                                    # start=True clears has_written for the WHOLE
                                    # bank: only the bank's first MM (h==0) may set
                                    # it, else h0's accumulation bits get wiped.
                                    # h1's first MM overwrites via cleared bits.
                                    nc.tensor.matmul(
                                        py[qi][:, h * DE : (h + 1) * DE],
                                        lhsT=esb[
                                            :,
                                            h * 512 + j * 256 + qi * 128 : h * 512 + j * 256 + qi * 128 + 128,
                                        ],
                                        rhs=v_sb[:, J, hg * DE : (hg + 1) * DE],
                                        start=(h == 0 and J == allowed_q[qi][0]),
                                        stop=(J == allowed_q[qi][-1]),
                                        skip_group_check=True,
                                    )
                    if debug and qp == 0 and hp == 0:
                        for qi in range(2):
                            pyc = outp.tile([128, 2 * DE], F32, name=f"pyc{qi}", tag="pyc")
                            nc.vector.tensor_copy(pyc, py[qi])
                            nc.sync.dma_start(out=dbg_py[qi], in_=pyc)
                    # normalize by rowsum, transpose to [d, q] layout
                    pyt = ps_t.tile([128, 256], BF16)
                    for qi in range(2):
                        for h in range(2):
                            rc = smallp.tile([128, 1], F32)
                            nc.vector.reciprocal(rc, py[qi][:, h * DE + D : h * DE + DE])
                            yn = ynp.tile([128, D], BF16)
                            nc.vector.tensor_scalar_mul(
                                yn, py[qi][:, h * DE : h * DE + D], rc
                            )
                            nc.tensor.transpose(
                                pyt[h * D : (h + 1) * D, ts(qi, 128)],
                                yn,
                                ident,
                                tile_position=(0, h * D),
                            )
                    yt = ytp.tile([128, 2, 128], BF16)
                    nc.vector.tensor_copy(
                        yt, pyt.rearrange("p (a q) -> p a q", a=2)
                    )
                    yts.append(yt)

                # output projection for the two query tiles
                for qi in range(2):
                    qt = q0 + qi
                    osb = outp.tile([128, C], F32)
                    for ch in range(2):
                        po = ps_o.tile([128, 512], F32, name="po", tag="po")
                        for hp in range(PAIRS):
                            nc.tensor.matmul(
                                po,
                                lhsT=yts[hp][:, qi, :],
                                rhs=wp_sb[:, hp, ts(ch, 512)],
                                start=(hp == 0),
                                stop=(hp == PAIRS - 1),
                            )
                        nc.vector.tensor_copy(osb[:, ts(ch, 512)], po)
                    nc.sync.dma_start(
                        out=outd[qt * 128 : (qt + 1) * 128, :], in_=osb
                    )

    nc.finalize()  # Bacc: runs compile pipeline (event-sem split, reg alloc)
    return nc


def _get_nc():
    if "nc" not in _CACHE:
        _CACHE["nc"] = _build()
    return _CACHE["nc"]


def _shard(inputs):
    bf = ml_dtypes.bfloat16
    x = np.asarray(inputs["x"], dtype=np.float32)
    Wq = np.asarray(inputs["Wq"], dtype=np.float32)
    Wk = np.asarray(inputs["Wk"], dtype=np.float32)
    Wv = np.asarray(inputs["Wv"], dtype=np.float32)
    Wp = np.asarray(inputs["Wp"], dtype=np.float32)
    bq = np.asarray(inputs["bq"], dtype=np.float32)
    bk = np.asarray(inputs["bk"], dtype=np.float32)
    bv = np.asarray(inputs["bv"], dtype=np.float32)

    in_maps = []
    for i in range(NCORE):
        b = i // 4
        j = i % 4
        hs = slice(j * CHS, (j + 1) * CHS)
        m = {
            "xT": np.ascontiguousarray(x[b].T).astype(bf),
            "wqT": np.ascontiguousarray(Wq[hs].T).astype(bf),
            "wkT": np.ascontiguousarray(Wk[hs].T).astype(bf),
            "wvT": np.ascontiguousarray(Wv[hs].T).astype(bf),
            "wpT": np.ascontiguousarray(Wp[:, hs].T).astype(bf),
            "bq": np.ascontiguousarray(bq[hs].reshape(PAIRS, 128).T),
            "bk": np.ascontiguousarray(bk[hs].reshape(PAIRS, 128).T),
            "bv": np.ascontiguousarray(np.broadcast_to(bv[hs], (128, CHS))),
        }
        in_maps.append(m)
    return in_maps


def _unshard(results, inputs):
    bp = np.asarray(inputs["bp"], dtype=np.float32)
    out = np.empty((B, T, C), dtype=np.float32)
    for b in range(B):
        acc = results[4 * b]["out"].astype(np.float32).copy()
        for j in range(1, 4):
            acc += results[4 * b + j]["out"]
        out[b] = acc + bp
    return out


def run(inputs, trace=False, debug=False):
    nc = _build(debug=True) if debug else _get_nc()
    in_maps = _shard(inputs)
    res = run_bass_kernel_spmd(nc, in_maps, list(range(NCORE)), trace=trace)
    return _unshard(res.results, inputs), res


def kernel(**inputs):
    out, _ = run(inputs, trace=False)
    return out


# revision 14
# speedup vs baseline: 1.2745x; 1.0059x over previous
"""Trainium2 Bass kernel for CausalSelfAttention with block-repeated causal mask.

Problem: B=2, T=3072, C=1024, H=16 heads, d=64.
  q/k/v = x @ W{q,k,v}.T + b;  scores = q k^T / 8, masked by
  (i % 1024) >= (j % 1024) (tril(1024) tiled 3x3), softmax, y = attn @ v,
  out = y @ Wp.T + bp.

Sharding (8 cores): core i handles batch b = i//4 and heads 4*(i%4)..4*(i%4)+3
(data parallel on B, tensor parallel on heads).  Each core computes a partial
output projection (its 4 heads' contribution, no bias); the host sums the 4
partials per batch and adds bp (the TP all-reduce done at unshard time).

Device layout per core (all matmul operands bf16, fp32 accumulation):
  xT   [C=1024, T]  : x[b] transposed (c_in on partitions)
  qT,kT [128, pair, T] : head-pair-packed [d-channel, t] projections
  v    [128, ktile, 4*(64+1)] : [t, head*(d | ones)] - ones col => rowsums
  scores^T tiles [k, q] via row-packed K=64 matmuls (2 heads concurrently)
  exp on ACT (scale=1/8 folded in), no max-subtraction (|scores| ~ 2)
  attn@v: lhsT = exp(s^T) tile, rhs = v_ext -> y psum [q, 65] accumulated
  normalize by reciprocal of rowsum col, PE-transpose, output projection.
"""

import numpy as np
import ml_dtypes

import concourse.bass as bass
from concourse import bacc
import concourse.mybir as mybir
from concourse.bass import ts
from concourse.tile import TileContext
from concourse.bass_utils import run_bass_kernel_spmd
from concourse.masks import make_identity, make_upper_triangular

B, T, C, H = 2, 3072, 1024, 16
D = 64                  # head dim
NCORE = 8
HPC = 4                 # heads per core
PAIRS = 2               # head pairs per core
CHS = HPC * D           # 256 channels per core
NKT = T // 128          # 24 key tiles
NQT = T // 128          # 24 query tiles
NQP = NQT // 2          # 12 query tile-pairs
RPB = 8                 # 128-tiles per 1024 mask block
DE = D + 1              # head value cols incl. ones column

BF16 = mybir.dt.bfloat16
F32 = mybir.dt.float32

_CACHE = {}


def _build(debug=False):
    nc = bacc.Bacc()

    xT = nc.dram_tensor("xT", [C, T], BF16, kind="ExternalInput")
    wqT = nc.dram_tensor("wqT", [C, CHS], BF16, kind="ExternalInput")
    wkT = nc.dram_tensor("wkT", [C, CHS], BF16, kind="ExternalInput")
    wvT = nc.dram_tensor("wvT", [C, CHS], BF16, kind="ExternalInput")
    wpT = nc.dram_tensor("wpT", [CHS, C], BF16, kind="ExternalInput")
    bqd = nc.dram_tensor("bq", [128, PAIRS], F32, kind="ExternalInput")
    bkd = nc.dram_tensor("bk", [128, PAIRS], F32, kind="ExternalInput")
    bvd = nc.dram_tensor("bv", [128, CHS], F32, kind="ExternalInput")
    outd = nc.dram_tensor("out", [T, C], F32, kind="ExternalOutput")
    if debug:
        dbg_qT = nc.dram_tensor("dbg_qT", [128, PAIRS, T], BF16, kind="ExternalOutput")
        dbg_kT = nc.dram_tensor("dbg_kT", [128, PAIRS, T], BF16, kind="ExternalOutput")
        dbg_v = nc.dram_tensor("dbg_v", [128, NKT, HPC * DE], BF16, kind="ExternalOutput")
        dbg_esb = nc.dram_tensor("dbg_esb", [128, 1024], BF16, kind="ExternalOutput")
        dbg_py = nc.dram_tensor("dbg_py", [2, 128, 2 * DE], F32, kind="ExternalOutput")

    with TileContext(nc) as tc:
        with (
            tc.tile_pool(name="const", bufs=1) as const,
            tc.tile_pool(name="qkv", bufs=1) as qkvp,
            tc.tile_pool(name="exps", bufs=6) as expp,
            tc.tile_pool(name="ynorm", bufs=8) as ynp,
            tc.tile_pool(name="ytp", bufs=6) as ytp,
            tc.tile_pool(name="outp", bufs=3) as outp,
            tc.tile_pool(name="small", bufs=16) as smallp,
            tc.tile_pool(name="ps_s", bufs=2, space="PSUM") as ps_s,
            tc.tile_pool(name="ps_y", bufs=2, space="PSUM") as ps_y,
            tc.tile_pool(name="ps_t", bufs=1, space="PSUM") as ps_t,
            tc.tile_pool(name="ps_o", bufs=1, space="PSUM") as ps_o,
        ):
            # ---------------- constants / weights into SBUF ----------------
            xT_sb = const.tile([128, C // 128, T], BF16)
            xT_ap = xT[:, :].rearrange("(a p) t -> a p t", p=128)
            for a in range(C // 128):
                nc.sync.dma_start(out=xT_sb[:, a, :], in_=xT_ap[a])

            wq_sb = const.tile([128, C // 128, CHS], BF16)
            wk_sb = const.tile([128, C // 128, CHS], BF16)
            wv_sb = const.tile([128, C // 128, CHS], BF16)
            for a in range(C // 128):
                nc.sync.dma_start(
                    out=wq_sb[:, a, :],
                    in_=wqT[:, :].rearrange("(a p) c -> a p c", p=128)[a],
                )
                nc.sync.dma_start(
                    out=wk_sb[:, a, :],
                    in_=wkT[:, :].rearrange("(a p) c -> a p c", p=128)[a],
                )
                nc.sync.dma_start(
                    out=wv_sb[:, a, :],
                    in_=wvT[:, :].rearrange("(a p) c -> a p c", p=128)[a],
                )
            wp_sb = const.tile([128, PAIRS, C], BF16)
            for a in range(PAIRS):
                nc.sync.dma_start(
                    out=wp_sb[:, a, :],
                    in_=wpT[:, :].rearrange("(a p) c -> a p c", p=128)[a],
                )

            bq_ld = const.tile([128, PAIRS], F32)
            bk_ld = const.tile([128, PAIRS], F32)
            bv_ld = const.tile([128, CHS], F32)
            nc.sync.dma_start(out=bq_ld, in_=bqd[:, :])
            nc.sync.dma_start(out=bk_ld, in_=bkd[:, :])
            nc.sync.dma_start(out=bv_ld, in_=bvd[:, :])
            # DVE-local copies: consumers then never need a DMA sem wait
            # (walrus allows only one sync-wait on TensorScalar/TensorTensor)
            bq_sb = const.tile([128, PAIRS], F32)
            bk_sb = const.tile([128, PAIRS], F32)
            bv_sb = const.tile([128, CHS], F32)
            nc.vector.tensor_copy(bq_sb, bq_ld)
            nc.vector.tensor_copy(bk_sb, bk_ld)
            nc.vector.tensor_copy(bv_sb, bv_ld)

            ident = const.tile([128, 128], BF16)
            make_identity(nc, ident)
            # mask[k', q'] = 1 where q' >= k' (keep), else 0
            mask_sb = const.tile([128, 128], BF16)
            make_upper_triangular(nc, mask_sb, val=1.0, diag=True)

            # ---------------- q/k/v projections ----------------
            qT_sb = qkvp.tile([128, PAIRS, T], BF16)
            # zero-padded per-head kT: full K=128 stationary for the scores
            # matmuls (rows outside the head's 64 are zero, multiplying the
            # other head's q rows by zero) - enables FWL and full-array MMs
            kTz = qkvp.tile([128, PAIRS * 2, T], BF16)
            v_sb = qkvp.tile([128, NKT, HPC * DE], BF16)
            nc.gpsimd.memset(kTz, 0.0)
            nc.vector.memset(v_sb, 1.0)  # ones columns for rowsums (DVE: keeps v bias-add single-wait)

            # qT/kT: psum[c_h(128 for the pair), t(512)] = sum_ci W^T . xT
            for pr in range(PAIRS):
                for tcn in range(T // 512):
                    pq = ps_s.tile([128, 1024], F32, name="pq", tag="sc")
                    for ci in range(C // 128):
                        nc.tensor.matmul(
                            pq[:, 0:512],
                            lhsT=wq_sb[:, ci, ts(pr, 128)],
                            rhs=xT_sb[:, ci, ts(tcn, 512)],
                            start=(ci == 0),
                            stop=(ci == C // 128 - 1),
                        )
                    for ci in range(C // 128):
                        nc.tensor.matmul(
                            pq[:, 512:1024],
                            lhsT=wk_sb[:, ci, ts(pr, 128)],
                            rhs=xT_sb[:, ci, ts(tcn, 512)],
                            start=(ci == 0),
                            stop=(ci == C // 128 - 1),
                        )
                    # bias add on DVE (TensorTensor takes multi-waits; the
                    # per-partition bias rides a free-broadcast AP), keeps ACT
                    # free for the exp softmax work
                    nc.vector.tensor_add(
                        qT_sb[:, pr, ts(tcn, 512)],
                        pq[:, 0:512],
                        bq_sb[:, pr : pr + 1].to_broadcast((128, 512)),
                    )
                    nc.vector.tensor_add(
                        kTz[0:D, pr * 2, ts(tcn, 512)],
                        pq[0:D, 512:1024],
                        bk_sb[0:D, pr : pr + 1].to_broadcast((D, 512)),
                    )
                    nc.vector.tensor_add(
                        kTz[D:128, pr * 2 + 1, ts(tcn, 512)],
                        pq[D:128, 512:1024],
                        bk_sb[D:128, pr : pr + 1].to_broadcast((D, 512)),
                    )

            # v: psum[t(128), c_h(256)] = sum_ci xT_tile^T . wvT
            bv_r = bv_sb.rearrange("p (h e) -> p h e", e=D)
            for th in range(NKT // 2):
                pv = ps_o.tile([128, 512], F32, name="pv", tag="po")
                for sub in range(2):
                    tt = th * 2 + sub
                    for ci in range(C // 128):
                        nc.tensor.matmul(
                            pv[:, ts(sub, 256)],
                            lhsT=xT_sb[:, ci, ts(tt, 128)],
                            rhs=wv_sb[:, ci, :],
                            start=(ci == 0),
                            stop=(ci == C // 128 - 1),
                        )
                for sub in range(2):
                    tt = th * 2 + sub
                    vt = v_sb[:, tt, :].rearrange("p (h e) -> p h e", e=DE)[:, :, 0:D]
                    pvr = pv[:, ts(sub, 256)].rearrange("p (h e) -> p h e", e=D)
                    nc.vector.tensor_add(vt, pvr, bv_r)

            if debug:
                nc.sync.dma_start(out=dbg_qT[:, :, :], in_=qT_sb)
                nc.sync.dma_start(out=dbg_kT[:, :, :], in_=kTz[:, 0:2, :])
                nc.sync.dma_start(out=dbg_v[:, :, :], in_=v_sb)

            # ---------------- attention + output projection ----------------
            for qp in range(NQP):
                ri0 = (2 * qp) % RPB
                ri1 = ri0 + 1
                q0 = 2 * qp
                allowed = [b * RPB + r for b in range(3) for r in range(ri1 + 1)]
                allowed_q = [
                    [j for j in allowed if j % RPB <= ri0],
                    allowed,
                ]
                groups = [allowed[i : i + 2] for i in range(0, len(allowed), 2)]

                yts = []
                for hp in range(PAIRS):
                    py = [
                        ps_y.tile([128, 2 * DE], F32, name=f"py{qi}", tag="py")
                        for qi in range(2)
                    ]
                    for g in groups:
                        pscore = ps_s.tile([128, 1024], F32, name="pscore", tag="sc")
                        # scores^T [k,q] : 2 heads in row groups 0-63 / 64-127
                        for j, J in enumerate(g):
                            for h in range(2):
                                nc.tensor.matmul(
                                    pscore[:, h * 512 + j * 256 : h * 512 + j * 256 + 256],
                                    lhsT=kTz[:, hp * 2 + h, ts(J, 128)],
                                    rhs=qT_sb[:, hp, q0 * 128 : q0 * 128 + 256],
                                    start=True,
                                    stop=True,
                                )
                        esb = expp.tile([128, 1024], BF16)
                        nc.scalar.activation(
                            esb, pscore, mybir.ActivationFunctionType.Exp, scale=0.125
                        )
                        # mask the two diagonal tile halves (on gpsimd, SBUF)
                        for j, J in enumerate(g):
                            r = J % RPB
                            if r in (ri0, ri1):
                                qi = 0 if r == ri0 else 1
                                for h in range(2):
                                    sl = esb[
                                        :,
                                        h * 512 + j * 256 + qi * 128 : h * 512 + j * 256 + qi * 128 + 128,
                                    ]
                                    nc.vector.tensor_mul(sl, sl, mask_sb)
                        if debug and qp == 0 and hp == 0 and g is groups[0]:
                            nc.sync.dma_start(out=dbg_esb[:, :], in_=esb)
                        # attn @ v_ext -> y psum [q, d|rowsum] accumulation
                        for j, J in enumerate(g):
                            r = J % RPB
                            for h in range(2):
                                hg = hp * 2 + h
                                for qi in range(2):
                                    if qi == 0 and r == ri1:
                                        continue
                                    # start=True clears has_written for the WHOLE
                                    # bank: only the bank's first MM (h==0) may set
                                    # it, else h0's accumulation bits get wiped.
                                    # h1's first MM overwrites via cleared bits.
                                    nc.tensor.matmul(
                                        py[qi][:, h * DE : (h + 1) * DE],
                                        lhsT=esb[
                                            :,
                                            h * 512 + j * 256 + qi * 128 : h * 512 + j * 256 + qi * 128 + 128,
                                        ],
                                        rhs=v_sb[:, J, hg * DE : (hg + 1) * DE],
                                        start=(h == 0 and J == allowed_q[qi][0]),
                                        stop=(J == allowed_q[qi][-1]),
                                        skip_group_check=True,
                                    )
                    if debug and qp == 0 and hp == 0:
                        for qi in range(2):
                            pyc = outp.tile([128, 2 * DE], F32, name=f"pyc{qi}", tag="pyc")
                            nc.vector.tensor_copy(pyc, py[qi])
                            nc.sync.dma_start(out=dbg_py[qi], in_=pyc)
                    # normalize by rowsum, transpose to [d, q] layout
                    pyt = ps_t.tile([128, 256], BF16)
                    for qi in range(2):
                        for h in range(2):
                            rc = smallp.tile([128, 1], F32)
                            nc.vector.reciprocal(rc, py[qi][:, h * DE + D : h * DE + DE])
                            yn = ynp.tile([128, D], BF16)
                            nc.vector.tensor_scalar_mul(
                                yn, py[qi][:, h * DE : h * DE + D], rc
                            )
                            nc.tensor.transpose(
                                pyt[h * D : (h + 1) * D, ts(qi, 128)],
                                yn,
                                ident,
                                tile_position=(0, h * D),
                            )
                    yt = ytp.tile([128, 2, 128], BF16)
                    nc.vector.tensor_copy(
                        yt, pyt.rearrange("p (a q) -> p a q", a=2)
                    )
                    yts.append(yt)

                # output projection for the two query tiles
                for qi in range(2):
                    qt = q0 + qi
                    osb = outp.tile([128, C], F32)
                    for ch in range(2):
                        po = ps_o.tile([128, 512], F32, name="po", tag="po")
                        for hp in range(PAIRS):
                            nc.tensor.matmul(
                                po,
                                lhsT=yts[hp][:, qi, :],
                                rhs=wp_sb[:, hp, ts(ch, 512)],
                                start=(hp == 0),
                                stop=(hp == PAIRS - 1),
                            )
                        nc.vector.tensor_copy(osb[:, ts(ch, 512)], po)
                    nc.sync.dma_start(
                        out=outd[qt * 128 : (qt + 1) * 128, :], in_=osb
                    )

    nc.finalize()  # Bacc: runs compile pipeline (event-sem split, reg alloc)
    return nc


def _get_nc():
    if "nc" not in _CACHE:
        _CACHE["nc"] = _build()
    return _CACHE["nc"]


def _shard(inputs):
    bf = ml_dtypes.bfloat16
    x = np.asarray(inputs["x"], dtype=np.float32)
    Wq = np.asarray(inputs["Wq"], dtype=np.float32)
    Wk = np.asarray(inputs["Wk"], dtype=np.float32)
    Wv = np.asarray(inputs["Wv"], dtype=np.float32)
    Wp = np.asarray(inputs["Wp"], dtype=np.float32)
    bq = np.asarray(inputs["bq"], dtype=np.float32)
    bk = np.asarray(inputs["bk"], dtype=np.float32)
    bv = np.asarray(inputs["bv"], dtype=np.float32)

    in_maps = []
    for i in range(NCORE):
        b = i // 4
        j = i % 4
        hs = slice(j * CHS, (j + 1) * CHS)
        m = {
            "xT": np.ascontiguousarray(x[b].T).astype(bf),
            "wqT": np.ascontiguousarray(Wq[hs].T).astype(bf),
            "wkT": np.ascontiguousarray(Wk[hs].T).astype(bf),
            "wvT": np.ascontiguousarray(Wv[hs].T).astype(bf),
            "wpT": np.ascontiguousarray(Wp[:, hs].T).astype(bf),
            "bq": np.ascontiguousarray(bq[hs].reshape(PAIRS, 128).T),
            "bk": np.ascontiguousarray(bk[hs].reshape(PAIRS, 128).T),
            "bv": np.ascontiguousarray(np.broadcast_to(bv[hs], (128, CHS))),
        }
        in_maps.append(m)
    return in_maps


def _unshard(results, inputs):
    bp = np.asarray(inputs["bp"], dtype=np.float32)
    out = np.empty((B, T, C), dtype=np.float32)
    for b in range(B):
        acc = results[4 * b]["out"].astype(np.float32).copy()
        for j in range(1, 4):
            acc += results[4 * b + j]["out"]
        out[b] = acc + bp
    return out


def run(inputs, trace=False, debug=False):
    nc = _build(debug=True) if debug else _get_nc()
    in_maps = _shard(inputs)
    res = run_bass_kernel_spmd(nc, in_maps, list(range(NCORE)), trace=trace)
    return _unshard(res.results, inputs), res


def kernel(**inputs):
    out, _ = run(inputs, trace=False)
    return out
